# revision 1
# baseline (speedup 1.0000x reference)
"""GAT layer (DiseaseGraphGAT) Trainium2 kernel, 8-way sharded over query rows.

Math (reference):
    s1 = emb @ attn[:D], s2 = emb @ attn[D:]          (N,)
    e  = leaky_relu(s1_i + s2_j, 0.2) masked by adj
    alpha = softmax(e, rows); out = alpha @ emb

Reformulation used here (per-row-scale invariant form; any positive per-i
factor cancels in the softmax ratio):
    w_ij / exp(s1_i) = exp(s2_j) * G_ij,  G_ij = exp(relu(-0.8*(s1_i+s2_j)))
    num_i = sum_j adj_ij * G_ij * E4_j      with E4 = diag(exp(s2)) @ emb
    Z_i   = sum_j adj_ij * G_ij * q4_j      with q4 = exp(s2)
    out_i = num_i / Z_i

Device pipeline per (128-row i-block, 2048-col j-strip), natural layout:
    1. HWDGE DMA: adj tile (int32) streams HBM->SBUF, prefetched a strip ahead
    2. DVE tensor_scalar:  r = max(S2B_scaled + bias_i, 0)       [f32]
       where S2B_scaled = -0.8*s2/256 broadcast, bias_i = -0.8*s1_i/256
    3. DVE tensor_tensor:  r += f32(adj)   (int32 converted on read)
    4. ACT:                aw = Exp(256*r - 256) -> bf16
       adj=1 -> exp(relu(-0.8 x)) ; adj=0 -> exp(...-256) == 0  (exact mask)
    5. xbar DMA transpose: batched per-128-block transposes into the AWT strip
       (3D out AP; j lands on partitions for the aggregation contraction)
    6. PE: psum_num[d,i] += E4_chunk.T @ AWT ; psum_z[0,i] += q4_chunk.T @ AWT

Host does the tiny O(N*D) precompute (s1, s2, E4) and the final divide.
Measured ~200-300 us/core on HW (repeat-differenced); cost-model sim 224 us;
pure adj-stream floor ~77-90 us.
"""

import sys

sys.path.insert(0, "/opt/trn_rl_repo")

import numpy as np
import ml_dtypes

import concourse.bacc as bacc
import concourse.mybir as mybir
import concourse.tile as tile
from concourse.bass_utils import run_bass_kernel_spmd

N = 8192
D = 128
NCORES = 8
NI_CORE = N // NCORES          # 1024 query rows per core
IBLK = 128                     # i-block (partition dim)
ICHUNK = 512                   # i extent per psum accumulation group
JSTRIP = 2048                  # j extent per build tile
WORK_BUFS = 3
TT_SPLIT = False
TS_ACT_SPLIT = False
R_BUFS = 6
AW_BUFS = 4
AD_BUFS = 4
AWT_BUFS = 2
NJC = N // 128                 # 64 j-chunks of 128
BIG = 256.0

_cache = {}


def _build_program(repeat=1, stages=("load","ts","tt","exp","tr","mm"), accum=False):
    key = ("nc", repeat, tuple(stages), accum, TT_SPLIT, TS_ACT_SPLIT)
    if key in _cache:
        return _cache[key]
    nc = bacc.Bacc("TRN2", target_bir_lowering=False, debug=False)
    adj_d = nc.declare_dram_parameter("adjs", [NI_CORE, N], mybir.dt.int32, isOutput=False)
    # packed preamble: cols [0:8]=per-i-block bias, col 8 = -BIG, cols 9: = -0.8*s2/BIG
    pre_d = nc.declare_dram_parameter("pre", [128, 9 + N], mybir.dt.float32, isOutput=False)
    e4_d = nc.declare_dram_parameter("e4", [128, NJC * D], mybir.dt.bfloat16, isOutput=False)
    q4_d = nc.declare_dram_parameter("q4", [128, NJC], mybir.dt.bfloat16, isOutput=False)
    numt_d = nc.declare_dram_parameter("numt", [D, NI_CORE], mybir.dt.float32, isOutput=True)
    z_d = nc.declare_dram_parameter("z", [1, NI_CORE], mybir.dt.float32, isOutput=True)

    NSTRIP = N // JSTRIP                # 4 j-strips
    JC_PER_STRIP = JSTRIP // 128        # 16 chunks per strip
    IB_PER_CHUNK = ICHUNK // IBLK       # 4 i-blocks per i-chunk
    NICHUNK = NI_CORE // ICHUNK         # 2 i-chunks per core

    with tile.TileContext(nc) as tc:
        with (
            tc.tile_pool(name="pre", bufs=1) as pre_pool,
            tc.tile_pool(name="workr", bufs=R_BUFS) as workr,
            tc.tile_pool(name="workaw", bufs=AW_BUFS) as workaw,
            tc.tile_pool(name="adp", bufs=AD_BUFS) as adp,
            tc.tile_pool(name="awt", bufs=AWT_BUFS) as awt_pool,
            tc.tile_pool(name="outp", bufs=2) as outp,
            tc.tile_pool(name="ps", bufs=2, space="PSUM") as ps,
        ):
          for _rep in range(repeat):
            pre = pre_pool.tile([128, 9 + N], mybir.dt.float32)
            nc.sync.dma_start(out=pre[:], in_=pre_d[:])
            e4 = pre_pool.tile([128, NJC * D], mybir.dt.bfloat16)
            nc.sync.dma_start(out=e4[:], in_=e4_d[:])
            q4 = pre_pool.tile([128, NJC], mybir.dt.bfloat16)
            nc.sync.dma_start(out=q4[:], in_=q4_d[:])
            nbias = pre[:, 8:9]

            def emit_loads(ic, js):
                out = []
                for ib in range(IB_PER_CHUNK):
                    gib = ic * IB_PER_CHUNK + ib
                    ad = adp.tile([IBLK, JSTRIP], mybir.dt.int32, tag="ad")
                    if "load" in stages:
                        nc.sync.dma_start(
                            out=ad[:],
                            in_=adj_d[gib * IBLK:(gib + 1) * IBLK,
                                      js * JSTRIP:(js + 1) * JSTRIP])
                    out.append(ad)
                return out

            pending = emit_loads(0, 0) if not accum else None
            for ic in range(NICHUNK):
                ps_num = ps.tile([D, ICHUNK], mybir.dt.float32, tag="psnum")
                ps_z = ps.tile([1, ICHUNK], mybir.dt.float32, tag="psz")
                for js in range(NSTRIP):
                    awt = awt_pool.tile([128, JC_PER_STRIP * ICHUNK], mybir.dt.bfloat16)
                    rs, aws = [], []
                    for ib in range(IB_PER_CHUNK):
                        gib = ic * IB_PER_CHUNK + ib   # global i-block in core
                        r = workr.tile([IBLK, JSTRIP], mybir.dt.float32, tag="r")
                        if "ts" in stages:
                            if TS_ACT_SPLIT and ib % 2 == 1:
                                nc.scalar.activation(
                                    r[:], pre[:, 9 + js * JSTRIP: 9 + (js + 1) * JSTRIP],
                                    mybir.ActivationFunctionType.Relu,
                                    bias=pre[:, gib:gib + 1], scale=1.0)
                            else:
                                nc.vector.tensor_scalar(
                                    r[:], pre[:, 9 + js * JSTRIP: 9 + (js + 1) * JSTRIP],
                                    pre[:, gib:gib + 1], 0.0,
                                    mybir.AluOpType.add, mybir.AluOpType.max)
                        rs.append(r)
                    if accum:
                        for ib in range(IB_PER_CHUNK):
                            gib = ic * IB_PER_CHUNK + ib
                            nc.gpsimd.dma_start(
                                out=rs[ib][:],
                                in_=adj_d[gib * IBLK:(gib + 1) * IBLK,
                                          js * JSTRIP:(js + 1) * JSTRIP],
                                accum_op=mybir.AluOpType.add)
                    else:
                        ads = pending
                        # prefetch next strip's adj while this strip computes
                        nic, njs = (ic, js + 1) if js + 1 < NSTRIP else (ic + 1, 0)
                        if nic < NICHUNK:
                            pending = emit_loads(nic, njs)
                        if "tt" in stages:
                            for ib in range(IB_PER_CHUNK):
                                eng = nc.vector if (ib % 2 == 0 or not TT_SPLIT) else nc.gpsimd
                                eng.tensor_tensor(rs[ib][:], ads[ib][:], rs[ib][:],
                                                  mybir.AluOpType.add)
                    for ib in range(IB_PER_CHUNK):
                        aw = workaw.tile([IBLK, JSTRIP], mybir.dt.bfloat16, tag="aw")
                        if "exp" in stages:
                            nc.scalar.activation(aw[:], rs[ib][:],
                                                 mybir.ActivationFunctionType.Exp,
                                                 bias=nbias, scale=BIG)
                        aws.append(aw)
                    if "tr" in stages:
                        for ib in range(IB_PER_CHUNK):
                            # scatter 16 per-128-block transposes into the AWT strip
                            out_3d = awt[:].rearrange("p (b q) -> p b q", b=JC_PER_STRIP)[
                                :, :, ib * IBLK:(ib + 1) * IBLK]
                            nc.sync.dma_start_transpose(out_3d, aws[ib][:])
                    if "mm" not in stages:
                        continue
                    for jc in range(JC_PER_STRIP):
                        g = js * JC_PER_STRIP + jc     # global j-chunk
                        first = (js == 0 and jc == 0)
                        last = (js == NSTRIP - 1 and jc == JC_PER_STRIP - 1)
                        rhs = awt[:, jc * ICHUNK:(jc + 1) * ICHUNK]
                        nc.tensor.matmul(ps_num[:], e4[:, g * D:(g + 1) * D], rhs,
                                         start=first, stop=last)
                        nc.tensor.matmul(ps_z[:], q4[:, g:g + 1], rhs,
                                         start=first, stop=last)
                if "mm" not in stages:
                    continue
                on = outp.tile([D, ICHUNK], mybir.dt.float32, tag="on")
                nc.vector.tensor_copy(on[:], ps_num[:])
                nc.sync.dma_start(out=numt_d[:, ic * ICHUNK:(ic + 1) * ICHUNK], in_=on[:])
                oz = outp.tile([1, ICHUNK], mybir.dt.float32, tag="oz")
                nc.scalar.copy(oz[:], ps_z[:])
                nc.sync.dma_start(out=z_d[:, ic * ICHUNK:(ic + 1) * ICHUNK], in_=oz[:])

    nc.compile()
    _cache[key] = nc
    return nc


def prep_in_maps(adj: np.ndarray, emb: np.ndarray, attn: np.ndarray) -> list:
    emb64 = emb.astype(np.float64)
    s1 = (emb64 @ attn[:D, 0].astype(np.float64)).astype(np.float32)
    s2 = (emb64 @ attn[D:, 0].astype(np.float64)).astype(np.float32)

    q4f = np.exp(s2.astype(np.float64)).astype(np.float32)       # exp(s2)
    e4f = (q4f[:, None] * emb)                                   # (N, D) f32
    # device layouts
    e4_dev = np.ascontiguousarray(
        e4f.reshape(NJC, 128, D).transpose(1, 0, 2).reshape(128, NJC * D)
    ).astype(ml_dtypes.bfloat16)
    q4_dev = np.ascontiguousarray(q4f.reshape(NJC, 128).T).astype(ml_dtypes.bfloat16)

    s2_scaled = (-0.8 / BIG) * s2

    in_maps = []
    for c in range(NCORES):
        rows = slice(c * NI_CORE, (c + 1) * NI_CORE)
        s1c = s1[rows]
        bias_cols = (-0.8 / BIG) * s1c.reshape(NI_CORE // IBLK, IBLK).T  # (128, 8)
        pre = np.empty((128, 9 + N), np.float32)
        pre[:, :8] = bias_cols
        pre[:, 8] = -BIG
        pre[:, 9:] = s2_scaled[None, :]
        in_maps.append({
            "adjs": np.ascontiguousarray(adj[rows]),
            "pre": pre,
            "e4": e4_dev,
            "q4": q4_dev,
        })
    return in_maps


def kernel(adj: np.ndarray, emb: np.ndarray, attn: np.ndarray) -> np.ndarray:
    in_maps = prep_in_maps(adj, emb, attn)
    nc = _build_program()
    res = run_bass_kernel_spmd(nc, in_maps, core_ids=list(range(NCORES)))

    out = np.empty((N, D), np.float32)
    for c, r in enumerate(res.results):
        numt = r["numt"]          # (D, NI_CORE)
        z = r["z"]                # (1, NI_CORE)
        out[c * NI_CORE:(c + 1) * NI_CORE] = (numt / z).T
    return out



# revision 3
# speedup vs baseline: 1.5503x; 1.5503x over previous
"""GAT layer (DiseaseGraphGAT) Trainium2 kernel, 8-way sharded over query rows.

Math (reference):
    s1 = emb @ attn[:D], s2 = emb @ attn[D:]          (N,)
    e  = leaky_relu(s1_i + s2_j, 0.2) masked by adj
    alpha = softmax(e, rows); out = alpha @ emb

Rank-1 reformulation (per-row-scale invariant; any positive per-i factor
cancels in the softmax ratio):
    exp(relu(-0.8(s1_i+s2_j))) = max(1, a_i*u_j),  a=exp(-0.8 s1), u=exp(-0.8 s2)
    w_ij = adj_ij * max(1, a_i*u_j) * q4_j,        q4 = exp(s2)
    out_i = (sum_j w_ij emb_j) / (sum_j w_ij)

Device pipeline per (128-row i-block, 2048-col j-strip), all bf16:
    1. HWDGE DMA: adjq tile (bf16, = adj*q4 precomputed on host) HBM->SBUF
    2. DVE tensor_scalar (4x):       m  = max(u_strip * a_i, 1)
    3. DVE tensor_tensor_reduce(2x): aw = m * adjq ; zpart += row-sum(aw)
    4. xbar DMA transpose: per-128-block transposes into the AWT strip
    5. PE: psum_num[d,i] += emb_chunk.T @ AWT     (no separate Z matmul)

Host does the tiny O(N*D) precompute (s1, s2, a, u, q4) and the final
divide. The exp is rank-1 so no ACT activation pass is needed at all;
elementwise work is 2 bf16 DVE instructions per tile.
"""

import sys

sys.path.insert(0, "/opt/trn_rl_repo")

import numpy as np
import ml_dtypes

import concourse.bacc as bacc
import concourse.mybir as mybir
import concourse.tile as tile
from concourse.bass_utils import run_bass_kernel_spmd

N = 8192
D = 128
NCORES = 8
NI_CORE = N // NCORES          # 1024 query rows per core
IBLK = 128                     # i-block (partition dim)
ICHUNK = 512                   # i extent per psum accumulation group
JSTRIP = 2048                  # j extent per build tile
NJC = N // 128                 # 64 j-chunks of 128
NSTRIP = N // JSTRIP           # 4 j-strips
JC_PER_STRIP = JSTRIP // 128   # 16 chunks per strip
IB_PER_CHUNK = ICHUNK // IBLK  # 4 i-blocks per i-chunk
NICHUNK = NI_CORE // ICHUNK    # 2 i-chunks per core
NIB = NI_CORE // IBLK          # 8 i-blocks per core

M_BUFS = 4
AW_BUFS = 4
AD_BUFS = 8
AWT_BUFS = 2

_cache = {}


def _build_program(repeat=1):
    key = ("nc", repeat)
    if key in _cache:
        return _cache[key]
    nc = bacc.Bacc("TRN2", target_bir_lowering=False, debug=False)
    adjq_d = nc.declare_dram_parameter("adjq", [NI_CORE, N], mybir.dt.bfloat16, isOutput=False)
    aux_d = nc.declare_dram_parameter("aux", [128, NIB], mybir.dt.float32, isOutput=False)
    ub_d = nc.declare_dram_parameter("ub", [128, N], mybir.dt.bfloat16, isOutput=False)
    embc_d = nc.declare_dram_parameter("embc", [128, NJC * D], mybir.dt.bfloat16, isOutput=False)
    numt_d = nc.declare_dram_parameter("numt", [D, NI_CORE], mybir.dt.float32, isOutput=True)
    z_d = nc.declare_dram_parameter("z", [1, NI_CORE], mybir.dt.float32, isOutput=True)

    with tile.TileContext(nc) as tc:
        with (
            tc.tile_pool(name="pre", bufs=1) as pre_pool,
            tc.tile_pool(name="workm", bufs=M_BUFS) as workm,
            tc.tile_pool(name="workaw", bufs=AW_BUFS) as workaw,
            tc.tile_pool(name="adp", bufs=AD_BUFS) as adp,
            tc.tile_pool(name="awt", bufs=AWT_BUFS) as awt_pool,
            tc.tile_pool(name="outp", bufs=2) as outp,
            tc.tile_pool(name="ps", bufs=2, space="PSUM") as ps,
        ):
          for _rep in range(repeat):
            aux = pre_pool.tile([128, NIB], mybir.dt.float32)
            nc.sync.dma_start(out=aux[:], in_=aux_d[:])
            ub = pre_pool.tile([128, N], mybir.dt.bfloat16)
            nc.sync.dma_start(out=ub[:], in_=ub_d[:])
            embc = pre_pool.tile([128, NJC * D], mybir.dt.bfloat16)
            nc.sync.dma_start(out=embc[:], in_=embc_d[:])
            ones = pre_pool.tile([128, 1], mybir.dt.bfloat16)
            nc.vector.memset(ones[:], 1.0)

            def emit_loads(ic, js):
                out = []
                for ib in range(IB_PER_CHUNK):
                    gib = ic * IB_PER_CHUNK + ib
                    ad = adp.tile([IBLK, JSTRIP], mybir.dt.bfloat16, tag="ad")
                    nc.sync.dma_start(
                        out=ad[:],
                        in_=adjq_d[gib * IBLK:(gib + 1) * IBLK,
                                   js * JSTRIP:(js + 1) * JSTRIP])
                    out.append(ad)
                return out

            pending = emit_loads(0, 0)
            for ic in range(NICHUNK):
                ps_num = ps.tile([D, ICHUNK], mybir.dt.float32, tag="psnum")
                ps_z = ps.tile([1, ICHUNK], mybir.dt.float32, tag="psz")
                for js in range(NSTRIP):
                    awt = awt_pool.tile([128, JC_PER_STRIP * ICHUNK], mybir.dt.bfloat16)
                    ads = pending
                    # prefetch next strip's adjq while this strip computes
                    nic, njs = (ic, js + 1) if js + 1 < NSTRIP else (ic + 1, 0)
                    if nic < NICHUNK:
                        pending = emit_loads(nic, njs)
                    for ib in range(IB_PER_CHUNK):
                        gib = ic * IB_PER_CHUNK + ib   # global i-block in core
                        m = workm.tile([IBLK, JSTRIP], mybir.dt.bfloat16, tag="m")
                        nc.vector.tensor_scalar(
                            m[:], ub[:, js * JSTRIP:(js + 1) * JSTRIP],
                            aux[:, gib:gib + 1], 1.0,
                            mybir.AluOpType.mult, mybir.AluOpType.max)
                        aw = workaw.tile([IBLK, JSTRIP], mybir.dt.bfloat16, tag="aw")
                        nc.vector.tensor_tensor(aw[:], m[:], ads[ib][:],
                                                mybir.AluOpType.mult)
                        # scatter 16 per-128-block transposes into the AWT strip
                        out_3d = awt[:].rearrange("p (b q) -> p b q", b=JC_PER_STRIP)[
                            :, :, ib * IBLK:(ib + 1) * IBLK]
                        nc.sync.dma_start_transpose(out_3d, aw[:])
                    for jc in range(JC_PER_STRIP):
                        g = js * JC_PER_STRIP + jc     # global j-chunk
                        first = (js == 0 and jc == 0)
                        last = (js == NSTRIP - 1 and jc == JC_PER_STRIP - 1)
                        rhs = awt[:, jc * ICHUNK:(jc + 1) * ICHUNK]
                        nc.tensor.matmul(ps_num[:], embc[:, g * D:(g + 1) * D], rhs,
                                         start=first, stop=last)
                        nc.tensor.matmul(ps_z[:], ones[:], rhs,
                                         start=first, stop=last)
                on = outp.tile([D, ICHUNK], mybir.dt.float32, tag="on")
                nc.scalar.copy(on[:], ps_num[:])
                nc.sync.dma_start(out=numt_d[:, ic * ICHUNK:(ic + 1) * ICHUNK], in_=on[:])
                oz = outp.tile([1, ICHUNK], mybir.dt.float32, tag="oz")
                nc.scalar.copy(oz[:], ps_z[:])
                nc.sync.dma_start(out=z_d[:, ic * ICHUNK:(ic + 1) * ICHUNK], in_=oz[:])

    nc.compile()
    _cache[key] = nc
    return nc


def prep_in_maps(adj: np.ndarray, emb: np.ndarray, attn: np.ndarray) -> list:
    emb64 = emb.astype(np.float64)
    s1 = emb64 @ attn[:D, 0].astype(np.float64)
    s2 = emb64 @ attn[D:, 0].astype(np.float64)

    a = np.exp(-0.8 * s1).astype(np.float32)            # (N,)
    u = np.exp(-0.8 * s2).astype(ml_dtypes.bfloat16)    # (N,)
    q4 = np.exp(s2).astype(ml_dtypes.bfloat16)          # (N,)

    # adjq = adj * q4 in bf16 via integer trick: adj is 0/1 so the bf16 bit
    # pattern of adj*q4 is adj * bitpattern(q4)
    q4u = q4.view(np.uint16)
    adjq_u = adj.astype(np.uint16) * q4u[None, :]       # (N, N) uint16
    adjq = adjq_u.view(ml_dtypes.bfloat16)

    ub = np.ascontiguousarray(np.broadcast_to(u[None, :], (128, N)))
    embc = np.ascontiguousarray(
        emb.reshape(NJC, 128, D).transpose(1, 0, 2).reshape(128, NJC * D)
    ).astype(ml_dtypes.bfloat16)

    in_maps = []
    for c in range(NCORES):
        rows = slice(c * NI_CORE, (c + 1) * NI_CORE)
        aux = np.ascontiguousarray(
            a[rows].reshape(NIB, IBLK).T)               # (128, 8)
        in_maps.append({
            "adjq": np.ascontiguousarray(adjq[rows]),
            "aux": aux,
            "ub": ub,
            "embc": embc,
        })
    return in_maps


def kernel(adj: np.ndarray, emb: np.ndarray, attn: np.ndarray) -> np.ndarray:
    in_maps = prep_in_maps(adj, emb, attn)
    nc = _build_program()
    res = run_bass_kernel_spmd(nc, in_maps, core_ids=list(range(NCORES)))

    out = np.empty((N, D), np.float32)
    for c, r in enumerate(res.results):
        numt = r["numt"]                                # (D, NI_CORE)
        z = r["z"]                                      # (1, NI_CORE)
        out[c * NI_CORE:(c + 1) * NI_CORE] = (numt / z).T
    return out


# revision 6
# speedup vs baseline: 4.4740x; 2.8858x over previous
"""GAT layer (DiseaseGraphGAT) Trainium2 kernel, 8-way sharded over query rows.

Math (reference):
    s1 = emb @ attn[:D], s2 = emb @ attn[D:]          (N,)
    e  = leaky_relu(s1_i + s2_j, 0.2) masked by adj
    alpha = softmax(e, rows); out = alpha @ emb

Rank-1 reformulation (per-row-scale invariant; any positive per-i factor
cancels in the softmax ratio):
    exp(relu(-0.8(s1_i+s2_j))) = max(1, a_i*u_j),  a=exp(-0.8 s1), u=exp(-0.8 s2)
    w_ij = adj_ij * max(1, a_i*u_j) * q4_j,        q4 = exp(s2)
    out_i = (sum_j w_ij emb_j) / (sum_j w_ij)

The adjacency ships pre-transposed AND pre-scaled from the host
(adjqT[j,i] = adj[i,j]*q4[j], bf16), so j lands on partitions directly and
the device needs NO xbar transpose (the xbar was the serial bottleneck of
the transpose-based variant: 14ns per 16x128 tile = ~57us/core).

Device pipeline per j-chunk g (tile [128j, 1024i], all bf16):
    1. HWDGE DMA: adjqT tile HBM->SBUF, prefetched ahead
    2. DVE tensor_scalar (4x):  mT = max(a_bcast * u_g, 1)
    3. DVE/GPSIMD tensor_tensor (2x): aw = mT * adjqT
    4. PE: ps_num[ic] += embc_g.T @ aw[:, ic];  ps_z[ic] += ones.T @ aw[:, ic]

Host does the tiny O(N*D) precompute (s1, s2, a, u) plus the O(N^2)
adj*q4 transpose/cast, and the final divide.
"""

import sys

sys.path.insert(0, "/opt/trn_rl_repo")

import numpy as np
import ml_dtypes

import concourse.bacc as bacc
import concourse.mybir as mybir
import concourse.tile as tile
from concourse.bass_utils import run_bass_kernel_spmd

N = 8192
D = 128
NCORES = 8
NI_CORE = N // NCORES          # 1024 query rows per core
ICHUNK = 512                   # i extent per psum tile
NIC = NI_CORE // ICHUNK        # 2 psum groups
NJC = N // 128                 # 64 j-chunks of 128

AD_BUFS = 8
M_BUFS = 4
AW_BUFS = 4
GTT_MOD = 0                    # every GTT_MOD-th tensor_tensor goes to GPSIMD (0=off)

_cache = {}


def _build_program(repeat=1, stages=("load", "ts", "tt", "mm"), gtt_mod=GTT_MOD):
    key = ("nc", repeat, tuple(stages), gtt_mod)
    if key in _cache:
        return _cache[key]
    nc = bacc.Bacc("TRN2", target_bir_lowering=False, debug=False)
    adjqt_d = nc.declare_dram_parameter("adjqt", [N, NI_CORE], mybir.dt.bfloat16, isOutput=False)
    ucols_d = nc.declare_dram_parameter("ucols", [128, NJC], mybir.dt.float32, isOutput=False)
    ab_d = nc.declare_dram_parameter("ab", [128, NI_CORE], mybir.dt.bfloat16, isOutput=False)
    embc_d = nc.declare_dram_parameter("embc", [128, NJC * D], mybir.dt.bfloat16, isOutput=False)
    numt_d = nc.declare_dram_parameter("numt", [D, NI_CORE], mybir.dt.float32, isOutput=True)
    z_d = nc.declare_dram_parameter("z", [1, NI_CORE], mybir.dt.float32, isOutput=True)

    with tile.TileContext(nc) as tc:
        with (
            tc.tile_pool(name="pre", bufs=1) as pre_pool,
            tc.tile_pool(name="workm", bufs=M_BUFS) as workm,
            tc.tile_pool(name="workaw", bufs=AW_BUFS) as workaw,
            tc.tile_pool(name="adp", bufs=AD_BUFS) as adp,
            tc.tile_pool(name="outp", bufs=2) as outp,
            tc.tile_pool(name="ps", bufs=1, space="PSUM") as ps,
        ):
          for _rep in range(repeat):
            ucols = pre_pool.tile([128, NJC], mybir.dt.float32)
            nc.sync.dma_start(out=ucols[:], in_=ucols_d[:])
            ab = pre_pool.tile([128, NI_CORE], mybir.dt.bfloat16)
            nc.sync.dma_start(out=ab[:], in_=ab_d[:])
            embc = pre_pool.tile([128, NJC * D], mybir.dt.bfloat16)
            nc.sync.dma_start(out=embc[:], in_=embc_d[:])
            ones = pre_pool.tile([128, 1], mybir.dt.bfloat16)
            nc.vector.memset(ones[:], 1.0)

            def emit_load(g):
                ad = adp.tile([128, NI_CORE], mybir.dt.bfloat16, tag="ad")
                if "load" in stages:
                    nc.sync.dma_start(out=ad[:], in_=adjqt_d[g * 128:(g + 1) * 128, :])
                return ad

            PREF = min(AD_BUFS - 1, 4)
            pending = [emit_load(g) for g in range(PREF)]
            ps_num = [ps.tile([D, ICHUNK], mybir.dt.float32, tag=f"psn{ic}",
                              name=f"psn{ic}") for ic in range(NIC)]
            ps_z = [ps.tile([1, ICHUNK], mybir.dt.float32, tag=f"psz{ic}",
                            name=f"psz{ic}") for ic in range(NIC)]
            for g in range(NJC):
                ad = pending.pop(0)
                if g + PREF < NJC:
                    pending.append(emit_load(g + PREF))
                m = workm.tile([128, NI_CORE], mybir.dt.bfloat16, tag="m")
                if "ts" in stages:
                    nc.vector.tensor_scalar(
                        m[:], ab[:], ucols[:, g:g + 1], 1.0,
                        mybir.AluOpType.mult, mybir.AluOpType.max)
                aw = workaw.tile([128, NI_CORE], mybir.dt.bfloat16, tag="aw")
                if "tt" in stages:
                    eng = nc.gpsimd if (gtt_mod and g % gtt_mod == gtt_mod - 1) else nc.vector
                    eng.tensor_tensor(aw[:], m[:], ad[:], mybir.AluOpType.mult)
                if "mm" not in stages:
                    continue
                first = (g == 0)
                last = (g == NJC - 1)
                for ic in range(NIC):
                    rhs = aw[:, ic * ICHUNK:(ic + 1) * ICHUNK]
                    nc.tensor.matmul(ps_num[ic][:], embc[:, g * D:(g + 1) * D], rhs,
                                     start=first, stop=last)
                    nc.tensor.matmul(ps_z[ic][:], ones[:], rhs,
                                     start=first, stop=last)
            if "mm" in stages:
                for ic in range(NIC):
                    on = outp.tile([D, ICHUNK], mybir.dt.float32, tag="on")
                    nc.scalar.copy(on[:], ps_num[ic][:])
                    nc.sync.dma_start(out=numt_d[:, ic * ICHUNK:(ic + 1) * ICHUNK], in_=on[:])
                    oz = outp.tile([1, ICHUNK], mybir.dt.float32, tag="oz")
                    nc.scalar.copy(oz[:], ps_z[ic][:])
                    nc.sync.dma_start(out=z_d[:, ic * ICHUNK:(ic + 1) * ICHUNK], in_=oz[:])

    nc.compile()
    _cache[key] = nc
    return nc


def prep_in_maps(adj: np.ndarray, emb: np.ndarray, attn: np.ndarray) -> list:
    emb64 = emb.astype(np.float64)
    s1 = emb64 @ attn[:D, 0].astype(np.float64)
    s2 = emb64 @ attn[D:, 0].astype(np.float64)

    a = np.exp(-0.8 * s1).astype(ml_dtypes.bfloat16)    # (N,)
    u = np.exp(-0.8 * s2).astype(np.float32)            # (N,)
    q4 = np.exp(s2).astype(ml_dtypes.bfloat16)          # (N,)

    # adjqT[j, i] = adj[i, j] * q4[j] in bf16 via integer trick: adj is 0/1
    # so the bf16 bit pattern of adj*q4 is adj * bitpattern(q4)
    q4u = q4.view(np.uint16)

    ucols = np.ascontiguousarray(u.reshape(NJC, 128).T)  # (128, NJC) f32
    embc = np.ascontiguousarray(
        emb.reshape(NJC, 128, D).transpose(1, 0, 2).reshape(128, NJC * D)
    ).astype(ml_dtypes.bfloat16)

    in_maps = []
    for c in range(NCORES):
        rows = slice(c * NI_CORE, (c + 1) * NI_CORE)
        adjqt_u = np.ascontiguousarray(adj[rows].T).astype(np.uint16) * q4u[:, None]
        ab = np.ascontiguousarray(a[None, rows])
        ab = np.ascontiguousarray(np.broadcast_to(ab, (128, NI_CORE)))
        in_maps.append({
            "adjqt": adjqt_u.view(ml_dtypes.bfloat16),
            "ucols": ucols,
            "ab": ab,
            "embc": embc,
        })
    return in_maps


def kernel(adj: np.ndarray, emb: np.ndarray, attn: np.ndarray) -> np.ndarray:
    in_maps = prep_in_maps(adj, emb, attn)
    nc = _build_program()
    res = run_bass_kernel_spmd(nc, in_maps, core_ids=list(range(NCORES)))

    out = np.empty((N, D), np.float32)
    for c, r in enumerate(res.results):
        numt = r["numt"]                                # (D, NI_CORE)
        z = r["z"]                                      # (1, NI_CORE)
        out[c * NI_CORE:(c + 1) * NI_CORE] = (numt / z).T
    return out
